# revision 53
# baseline (speedup 1.0000x reference)
"""Trainium2 Bass kernel for nn_BroadcastRouter (GNN message passing).

Computation (per region r of R=4096, B=16, D=256, N=16 neighbors, top-K=4):
  sims[r, n]  = mean over B*D of feats[r] * feats[nbr[r, n]]
  sel         = indices of top-4 sims (stable, jax.lax.top_k tie-breaking)
  agg[r]      = mean_k bcast[nbr[r, sel_k]]
  out[r]      = concat([bcast[r], agg[r]]) @ mix_w.T + mix_b

Distribution: regions sharded across 8 cores (512 each); the full feats/bcast
tables are replicated into every core's HBM so neighbor gathers are local
indirect DMAs (no collectives needed).

The fp32 version of this kernel is DMA-gather-bound (row gathers are ~87%
of HBM traffic), so all tables are stored fp16: gathered bytes halve vs
fp32. fp16 (not bf16) keeps the top-4 selection nearly exact — sims dots
accumulate in fp32 on DVE, and the only error is input rounding (4/4096
regions flip selection on this data, ~1e-2 final l2; bf16 would flip 22).
After the dtype switch the kernel is three-way balanced: DVE sims
multiply-reduce ~317us, DMA ~319us, gpsimd descriptor-gen ~293us per core.

Per-core pipeline, blocks of 128 regions; C/D phases lag two blocks behind
A/B so the bcast-gather descriptor generation never waits on the freshly
computed selection, and the last A/B segment absorbs two lagging C/D pairs
to shorten the epilogue:
  A: load local rows; 16x indirect row-gather of neighbor feats (fp16);
     one fused full-row multiply+accumulate (STT) per candidate on DVE
     -> sims [128, 16] fp32
  B: exact stable rank of each candidate (counting comparisons) -> top-4
  C: gather the 4 selected bcast rows (fp16), sum on DVE (0.25 folded into
     weights)
  D: PE-transpose agg in (ec, b, r) layout so each (b, ec) transpose feeds
     its own per-b mix matmul immediately; the local-bcast side is loaded
     pre-transposed from a host-prepped tensor (no PE work). Bias via a K=1
     matmul; fp16 output DMA'd on the ACT HWDGE ring, upcast on host.

All auxiliary constants (identity, ones row, tie-break mask) are host-built
inputs so the gpsimd engine's first instruction is gather descriptor-gen;
the bcast-row adds are emitted a full STT stream after their gathers so the
descriptor ring has always drained by the time DVE reaches them.

Each block's bcast-gather group is emitted at the end of the PREVIOUS A/B
segment, so the descriptor-ring load is level (24 gathers/segment) instead
of piling 28 into the last segment; 16 gather buffers let the ring bank a
lead before the epilogue.

Measured on trn2 (8 cores): ~398us HW exec (fp32 baseline: 658us), output
l2 rel err 9.93e-3 (= the host-side numpy simulation of the same
arithmetic, bit-for-bit in which regions flip).
"""

import numpy as np

R, B, D, N, TOP_K = 4096, 16, 256, 16, 4
NCORES = 8
BD = B * D
P = 128
EC = D // P  # 2 e-chunks of 128 per half


def build(r_total=R, n_cores=NCORES, debug=False):
    import concourse.bass as bass
    import concourse.bacc as bacc
    import concourse.mybir as mybir
    import concourse.tile as tile

    f32 = mybir.dt.float32
    f16 = mybir.dt.float16
    i32 = mybir.dt.int32
    i16 = mybir.dt.int16
    Alu = mybir.AluOpType

    rl = r_total // n_cores
    assert rl % P == 0
    blocks = [(i * P, P) for i in range(rl // P)]
    nblk = len(blocks)

    nc = bacc.Bacc("TRN2", target_bir_lowering=False, debug=False,
                   num_devices=n_cores)
    feats = nc.dram_tensor("feats", [r_total, BD], f16, kind="ExternalInput")
    bcast = nc.dram_tensor("bcast", [r_total, BD], f16, kind="ExternalInput")
    featsL = nc.dram_tensor("feats_local", [rl, BD], f16, kind="ExternalInput")
    # host-pretransposed local bcast: [blk, p, (ec, b, r)]
    bcastLT = nc.dram_tensor("bcast_lt", [nblk * P, EC * B * P], f16,
                             kind="ExternalInput")
    nbrL = nc.dram_tensor("nbr_local", [rl, N], i32, kind="ExternalInput")
    # w1t/w2t are [e, d] = mix_w[:, :D].T and 0.25 * mix_w[:, D:].T
    w1t = nc.dram_tensor("w1t", [D, D], f16, kind="ExternalInput")
    w2t = nc.dram_tensor("w2t", [D, D], f16, kind="ExternalInput")
    biasw = nc.dram_tensor("biasw", [1, D], f16, kind="ExternalInput")
    # host-built constants: identity (PE transpose), ones row (bias matmul),
    # strict-lower-triangular tie-break mask. Loading these as plain DMAs
    # keeps the gpsimd engine free to start gather descgen immediately.
    identw = nc.dram_tensor("identw", [P, P], f16, kind="ExternalInput")
    onesw = nc.dram_tensor("onesw", [1, P], f16, kind="ExternalInput")
    ltmw = nc.dram_tensor("ltmw", [P, N * N], f32, kind="ExternalInput")
    outL = nc.dram_tensor("out_local", [rl * B, D], f16, kind="ExternalOutput")
    if debug:
        dbg_sims = nc.dram_tensor("dbg_sims", [rl, N], f32, kind="ExternalOutput")
        dbg_sel = nc.dram_tensor("dbg_sel", [rl, TOP_K], i32, kind="ExternalOutput")

    with tile.TileContext(nc) as tc:
        with (
            tc.tile_pool(name="const", bufs=1) as const,
            tc.tile_pool(name="big", bufs=16) as big,
            tc.tile_pool(name="blt", bufs=3) as bltp,
            tc.tile_pool(name="lt", bufs=2) as ltp,
            tc.tile_pool(name="small", bufs=3) as small,
            tc.tile_pool(name="outp", bufs=3) as outp,
            tc.tile_pool(name="psum", bufs=2, space="PSUM") as psum,
            tc.tile_pool(name="psmm", bufs=2, space="PSUM") as psmm,
            tc.tile_pool(name="pjunk", bufs=2) as pjunk,
        ):
            ident = const.tile([P, P], f16, tag="ident")
            nc.sync.dma_start(out=ident[:], in_=identw[:])
            ones1 = const.tile([1, P], f16, tag="ones")
            nc.sync.dma_start(out=ones1[:], in_=onesw[:])
            w1sb = const.tile([P, EC, D], f16, tag="w1")
            w2sb = const.tile([P, EC, D], f16, tag="w2")
            for ec in range(EC):
                nc.sync.dma_start(out=w1sb[:, ec, :], in_=w1t[ec * P:(ec + 1) * P, :])
                nc.sync.dma_start(out=w2sb[:, ec, :], in_=w2t[ec * P:(ec + 1) * P, :])
            bsb = const.tile([1, D], f16, tag="bias")
            nc.sync.dma_start(out=bsb[:], in_=biasw[:])
            ltm = const.tile([P, N * N], f32, tag="ltm")
            nc.sync.dma_start(out=ltm[:], in_=ltmw[:])

            st = [dict() for _ in range(nblk)]

            A_SPLIT = 8

            def phase_a1(blk):
                """loads + first chunk of neighbor-feature gathers + sims."""
                s = st[blk]
                r0, rp = blocks[blk]
                idx_t = small.tile([rp, N], i32, tag="idx")
                nc.sync.dma_start(out=idx_t[:], in_=nbrL[r0:r0 + rp, :])
                L_t = ltp.tile([rp, BD], f16, tag="lt")
                nc.sync.dma_start(out=L_t[:], in_=featsL[r0:r0 + rp, :])
                BLT = bltp.tile([P, EC * B * P], f16, tag="blt")
                nc.sync.dma_start(out=BLT[:], in_=bcastLT[blk * P:(blk + 1) * P, :])
                sims = small.tile([rp, N], f32, tag="sims")
                s.update(idx_t=idx_t, sims=sims, L_t=L_t, BLT=BLT,
                         r0=r0, rp=rp)
                gather_sims(blk, range(A_SPLIT))

            def gather_sims(blk, ns):
                s = st[blk]
                rp = s["rp"]
                for n in ns:
                    G = big.tile([rp, BD], f16, tag="big")
                    nc.gpsimd.indirect_dma_start(
                        out=G[:], out_offset=None, in_=feats[:],
                        in_offset=bass.IndirectOffsetOnAxis(
                            ap=s["idx_t"][:, n:n + 1], axis=0),
                    )
                    # product goes to PSUM scratch (keeps SBUF write ports
                    # free for the concurrent gather DMAs); one full-row
                    # fused multiply+accumulate per candidate -> sims col.
                    junkp = pjunk.tile([P, BD], f16, tag="junk")
                    nc.vector.scalar_tensor_tensor(
                        out=junkp[:rp], in0=G[:],
                        scalar=0.0, in1=s["L_t"][:],
                        op0=Alu.bypass, op1=Alu.mult,
                        accum_out=s["sims"][:, n:n + 1],
                    )

            def phase_a2(blk):
                """remaining gathers + sims."""
                gather_sims(blk, range(A_SPLIT, N))

            def phase_b(blk):
                """exact stable rank (jax.lax.top_k tie-break) -> selected idx."""
                s = st[blk]
                sims, idx_t, rp = s["sims"], s["idx_t"], s["rp"]
                nbrf = small.tile([rp, N], f32, tag="nbrf")
                nc.vector.tensor_copy(out=nbrf[:], in_=idx_t[:])
                cnt = small.tile([rp, N], f32, tag="cnt")
                cnteq = small.tile([rp, N], f32, tag="cnteq")
                junk16 = small.tile([rp, N], f32, tag="junk16")
                # full 16x16 comparison matrices via stride-0 broadcast APs:
                # cmp[r, p, q] = op(sims[r, q], sims[r, p]); row-sum over q.
                simq = sims[:, None, :].to_broadcast([rp, N, N])
                simp = sims[:, :, None].to_broadcast([rp, N, N])
                cmp_t = small.tile([rp, N * N], f32, tag="cmp")
                cmpv = cmp_t[:].rearrange("p (a b) -> p a b", a=N)
                nc.vector.tensor_tensor(out=cmpv, in0=simq, in1=simp,
                                        op=Alu.is_gt)
                nc.vector.tensor_reduce(out=cnt[:], in_=cmpv,
                                        axis=mybir.AxisListType.X, op=Alu.add)
                nc.vector.tensor_tensor(out=cmpv, in0=simq, in1=simp,
                                        op=Alu.is_equal)
                nc.vector.tensor_tensor(out=cmpv, in0=cmpv,
                                        in1=ltm[:rp].rearrange(
                                            "p (a b) -> p a b", a=N),
                                        op=Alu.mult)
                nc.vector.tensor_reduce(out=cnteq[:], in_=cmpv,
                                        axis=mybir.AxisListType.X, op=Alu.add)
                rank = small.tile([rp, N], f32, tag="rank")
                nc.vector.tensor_tensor(out=rank[:], in0=cnt[:], in1=cnteq[:],
                                        op=Alu.add)
                # sel_k = neighbor index whose rank == k (unique by construction)
                self_f = small.tile([rp, TOP_K], f32, tag="self")
                for k in range(TOP_K):
                    nc.vector.scalar_tensor_tensor(
                        out=junk16[:], in0=rank[:], scalar=float(k), in1=nbrf[:],
                        op0=Alu.is_equal, op1=Alu.mult,
                        accum_out=self_f[:, k:k + 1],
                    )
                sel_i = small.tile([rp, TOP_K], i32, tag="seli")
                nc.vector.tensor_copy(out=sel_i[:], in_=self_f[:])
                s.update(sel_i=sel_i)

            def phase_cg(blk):
                """gather the 4 selected bcast rows (descgen enqueued early,
                while the current block's sims STT stream covers the ring
                latency)."""
                s = st[blk]
                sel_i, rp = s["sel_i"], s["rp"]
                g40 = big.tile([rp, BD], f16, tag="big")
                g41 = big.tile([rp, BD], f16, tag="big")
                g42 = big.tile([rp, BD], f16, tag="big")
                g43 = big.tile([rp, BD], f16, tag="big")
                G4 = [g40, g41, g42, g43]
                for k in range(TOP_K):
                    nc.gpsimd.indirect_dma_start(
                        out=G4[k][:], out_offset=None, in_=bcast[:],
                        in_offset=bass.IndirectOffsetOnAxis(
                            ap=sel_i[:, k:k + 1], axis=0),
                    )
                s.update(G4=G4)

            def phase_ca(blk):
                """sum the gathered rows (0.25 folded in w2t); emitted after a
                full STT stream so the gathers have long drained the ring and
                DVE never waits here."""
                s = st[blk]
                G4 = s["G4"]
                AG = G4[0]
                for k in range(1, TOP_K):
                    nc.vector.tensor_tensor(out=AG[:], in0=AG[:],
                                            in1=G4[k][:], op=Alu.add)
                s.update(AG=AG)

            def phase_d(blk):
                """transpose agg, final mix matmuls + bias, write out."""
                s = st[blk]
                r0, AG, BLT, rp = s["r0"], s["AG"], s["BLT"], s["rp"]
                if debug:
                    nc.sync.dma_start(out=dbg_sims[r0:r0 + P, :], in_=s["sims"][:])
                    nc.sync.dma_start(out=dbg_sel[r0:r0 + P, :], in_=s["sel_i"][:])
                AGT = big.tile([P, EC * rp * B], f16, tag="big")
                agv = AGT[:].rearrange("p (ec b r) -> p ec b r", ec=EC, b=B)
                outv = outL[:].rearrange("(r b) d -> r b d", b=B)
                # per-b interleave: each b's transposes feed its own matmul
                # chain immediately, so the first out-store fires ~14 b's
                # earlier than with a transpose-all-then-matmul-all order.
                for b_i in range(B):
                    for ec in range(EC):
                        pt = psum.tile([P, P], f16, tag="tr")
                        off = b_i * D + ec * P
                        nc.tensor.transpose(out=pt[:, :rp],
                                            in_=AG[:rp, off:off + P],
                                            identity=ident[:rp, :rp])
                        nc.scalar.copy(out=agv[:, ec, b_i, :],
                                       in_=pt[:, :rp])
                    ps = psmm.tile([P, D], f32, tag="mm")
                    first = True
                    for srcT, wsb in ((BLT, w1sb), (AGT, w2sb)):
                        for ec in range(EC):
                            off = ec * (B * rp) + b_i * rp
                            lhsT = srcT[:, off:off + rp]
                            nc.tensor.matmul(out=ps[:rp], lhsT=lhsT,
                                             rhs=wsb[:, ec, :],
                                             start=first, stop=False)
                            first = False
                    nc.tensor.matmul(out=ps[:rp], lhsT=ones1[:1, :rp],
                                     rhs=bsb[:1, :], start=False, stop=True)
                    ot = outp.tile([P, D], f16, tag="ot")
                    nc.scalar.copy(out=ot[:rp], in_=ps[:rp])
                    # out-stores ride the ACT HWDGE ring so they never sit in
                    # front of the next block's loads on the sync ring.
                    nc.scalar.dma_start(out=outv[r0:r0 + rp, b_i, :],
                                        in_=ot[:rp])

            # software-pipelined emission, C/D lag two blocks behind A/B so
            # the bcast-gather descgen never waits on sel_i (which is long
            # ready), and the gpsimd queue stays saturated with feats gathers.
            ph = {"a1": phase_a1, "a2": phase_a2, "b": phase_b,
                  "cg": phase_cg, "ca": phase_ca, "d": phase_d}
            # ring load levelled 16/20/20/24 gathers per segment: each cg(b)
            # rides one segment after b(b) computes, each ca one segment after
            # its cg (ring fully drained), and the D phases spread one per
            # segment so the PE pile-up never lands in the epilogue.
            sched = [("a1", 0), ("a2", 0), ("b", 0),
                     ("a1", 1), ("cg", 0), ("a2", 1), ("b", 1),
                     ("a1", 2), ("cg", 1), ("ca", 0), ("a2", 2), ("b", 2),
                     ("d", 0), ("ca", 1),
                     ("a1", 3), ("cg", 2), ("d", 1), ("a2", 3), ("ca", 2),
                     ("b", 3), ("d", 2), ("cg", 3), ("ca", 3), ("d", 3)]
            assert nblk == 4
            for name, b in sched:
                ph[name](b)

    nc.compile()
    return nc


_CACHE = {}


def _get_nc():
    if "nc" not in _CACHE:
        _CACHE["nc"] = build()
    return _CACHE["nc"]


def _prep_in_maps(bcast_by_region, feats_by_region, neighbor_indices, mix_w,
                  mix_b):
    f2 = np.ascontiguousarray(
        np.asarray(feats_by_region, dtype=np.float32).reshape(R, BD)
    ).astype(np.float16)
    bc = np.ascontiguousarray(
        np.asarray(bcast_by_region, dtype=np.float32).reshape(R, BD)
    ).astype(np.float16)
    nbr = np.ascontiguousarray(np.asarray(neighbor_indices, dtype=np.int32))
    mw = np.asarray(mix_w, dtype=np.float32)
    mb = np.asarray(mix_b, dtype=np.float32)
    w1t = np.ascontiguousarray(mw[:, :D].T).astype(np.float16)
    w2t = (np.ascontiguousarray(mw[:, D:].T) * np.float32(1.0 / TOP_K)).astype(
        np.float16)
    biasw = np.ascontiguousarray(mb.reshape(1, D)).astype(np.float16)
    identw = np.eye(P, dtype=np.float16)
    onesw = np.ones((1, P), dtype=np.float16)
    # strict lower-triangular [a, b] -> 1 if b < a, replicated per partition
    tri = np.tril(np.ones((N, N), dtype=np.float32), -1).reshape(1, N * N)
    ltmw = np.ascontiguousarray(np.broadcast_to(tri, (P, N * N)))

    rl = R // NCORES
    nblk = rl // P
    in_maps = []
    for c in range(NCORES):
        bcl = bc[c * rl:(c + 1) * rl]
        # [blk, p, (ec, b, r)] layout for the matmul lhsT side
        bclt = np.ascontiguousarray(
            bcl.reshape(nblk, P, B, EC, P).transpose(0, 4, 3, 2, 1)
            .reshape(nblk * P, EC * B * P))
        in_maps.append({
            "feats": f2,
            "bcast": bc,
            "feats_local": np.ascontiguousarray(f2[c * rl:(c + 1) * rl]),
            "bcast_lt": bclt,
            "nbr_local": np.ascontiguousarray(nbr[c * rl:(c + 1) * rl]),
            "w1t": w1t,
            "w2t": w2t,
            "biasw": biasw,
            "identw": identw,
            "onesw": onesw,
            "ltmw": ltmw,
        })
    return in_maps


def run(in_maps, **kwargs):
    from concourse.bass_utils import run_bass_kernel_spmd

    nc = _get_nc()
    return run_bass_kernel_spmd(nc, in_maps, list(range(NCORES)), **kwargs)


def assemble(res):
    rl = R // NCORES
    return np.concatenate(
        [res.results[c]["out_local"].reshape(rl, B, D).astype(np.float32)
         for c in range(NCORES)],
        axis=0)


def kernel(bcast_by_region, feats_by_region, neighbor_indices, mix_w, mix_b):
    import os

    in_maps = _prep_in_maps(bcast_by_region, feats_by_region,
                            neighbor_indices, mix_w, mix_b)
    # NTFF tracing needs hooks this environment may not have; make sure a
    # stray BASS_TRACE env var can't break the plain execution path.
    prev = os.environ.get("BASS_NEVER_TRACE")
    os.environ["BASS_NEVER_TRACE"] = "1"
    try:
        res = run(in_maps)
    finally:
        if prev is None:
            os.environ.pop("BASS_NEVER_TRACE", None)
        else:
            os.environ["BASS_NEVER_TRACE"] = prev
    return assemble(res)


# revision 54
# speedup vs baseline: 1.0731x; 1.0731x over previous
"""Trainium2 Bass kernel for nn_BroadcastRouter (GNN message passing).

Computation (per region r of R=4096, B=16, D=256, N=16 neighbors, top-K=4):
  sims[r, n]  = mean over B*D of feats[r] * feats[nbr[r, n]]
  sel         = indices of top-4 sims (stable, jax.lax.top_k tie-breaking)
  agg[r]      = mean_k bcast[nbr[r, sel_k]]
  out[r]      = concat([bcast[r], agg[r]]) @ mix_w.T + mix_b

Distribution: regions sharded across 8 cores (512 each); the full feats/bcast
tables are replicated into every core's HBM so neighbor gathers are local
indirect DMAs (no collectives needed).

The fp32 version of this kernel is DMA-gather-bound (row gathers are ~87%
of HBM traffic), so all tables are stored fp16: gathered bytes halve vs
fp32. fp16 (not bf16) keeps the top-4 selection nearly exact — sims dots
accumulate in fp32 on DVE, and the only error is input rounding (4/4096
regions flip selection on this data, ~1e-2 final l2; bf16 would flip 22).
After the dtype switch the kernel is three-way balanced: DVE sims
multiply-reduce ~317us, DMA ~319us, gpsimd descriptor-gen ~293us per core.

Per-core pipeline, blocks of 128 regions; C/D phases lag two blocks behind
A/B so the bcast-gather descriptor generation never waits on the freshly
computed selection, and the last A/B segment absorbs two lagging C/D pairs
to shorten the epilogue:
  A: load local rows; 16x indirect row-gather of neighbor feats (fp16);
     one fused full-row multiply+accumulate (STT) per candidate on DVE
     -> sims [128, 16] fp32
  B: exact stable rank of each candidate (counting comparisons) -> top-4
  C: gather the 4 selected bcast rows (fp16), sum on DVE (0.25 folded into
     weights)
  D: PE-transpose agg in (ec, b, r) layout so each (b, ec) transpose feeds
     its own per-b mix matmul immediately; the local-bcast side is loaded
     pre-transposed from a host-prepped tensor (no PE work). Bias via a K=1
     matmul; fp16 output DMA'd on the ACT HWDGE ring, upcast on host.

All auxiliary constants (identity, ones row, tie-break mask) are host-built
inputs so the gpsimd engine's first instruction is gather descriptor-gen;
the bcast-row adds are emitted a full STT stream after their gathers so the
descriptor ring has always drained by the time DVE reaches them.

Each block's bcast-gather group is emitted at the end of the PREVIOUS A/B
segment, so the descriptor-ring load is level (24 gathers/segment) instead
of piling 28 into the last segment; 16 gather buffers let the ring bank a
lead before the epilogue.

Measured on trn2 (8 cores): ~398us HW exec (fp32 baseline: 658us), output
l2 rel err 9.93e-3 (= the host-side numpy simulation of the same
arithmetic, bit-for-bit in which regions flip).
"""

import numpy as np

R, B, D, N, TOP_K = 4096, 16, 256, 16, 4
NCORES = 8
BD = B * D
P = 128
EC = D // P  # 2 e-chunks of 128 per half


def build(r_total=R, n_cores=NCORES, debug=False):
    import concourse.bass as bass
    import concourse.bacc as bacc
    import concourse.mybir as mybir
    import concourse.tile as tile

    f32 = mybir.dt.float32
    f16 = mybir.dt.float16
    i32 = mybir.dt.int32
    i16 = mybir.dt.int16
    Alu = mybir.AluOpType

    rl = r_total // n_cores
    assert rl % P == 0
    blocks = [(i * P, P) for i in range(rl // P)]
    nblk = len(blocks)

    nc = bacc.Bacc("TRN2", target_bir_lowering=False, debug=False,
                   num_devices=n_cores)
    feats = nc.dram_tensor("feats", [r_total, BD], f16, kind="ExternalInput")
    bcast = nc.dram_tensor("bcast", [r_total, BD], f16, kind="ExternalInput")
    featsL = nc.dram_tensor("feats_local", [rl, BD], f16, kind="ExternalInput")
    # host-pretransposed local bcast: [blk, p, (ec, b, r)]
    bcastLT = nc.dram_tensor("bcast_lt", [nblk * P, EC * B * P], f16,
                             kind="ExternalInput")
    nbrL = nc.dram_tensor("nbr_local", [rl, N], i32, kind="ExternalInput")
    # w1t/w2t are [e, d] = mix_w[:, :D].T and 0.25 * mix_w[:, D:].T
    w1t = nc.dram_tensor("w1t", [D, D], f16, kind="ExternalInput")
    w2t = nc.dram_tensor("w2t", [D, D], f16, kind="ExternalInput")
    biasw = nc.dram_tensor("biasw", [1, D], f16, kind="ExternalInput")
    # host-built constants: identity (PE transpose), ones row (bias matmul),
    # strict-lower-triangular tie-break mask. Loading these as plain DMAs
    # keeps the gpsimd engine free to start gather descgen immediately.
    identw = nc.dram_tensor("identw", [P, P], f16, kind="ExternalInput")
    onesw = nc.dram_tensor("onesw", [1, P], f16, kind="ExternalInput")
    ltmw = nc.dram_tensor("ltmw", [P, N * N], f32, kind="ExternalInput")
    outL = nc.dram_tensor("out_local", [rl * B, D], f16, kind="ExternalOutput")
    if debug:
        dbg_sims = nc.dram_tensor("dbg_sims", [rl, N], f32, kind="ExternalOutput")
        dbg_sel = nc.dram_tensor("dbg_sel", [rl, TOP_K], i32, kind="ExternalOutput")

    with tile.TileContext(nc) as tc:
        with (
            tc.tile_pool(name="const", bufs=1) as const,
            tc.tile_pool(name="big", bufs=16) as big,
            tc.tile_pool(name="blt", bufs=3) as bltp,
            tc.tile_pool(name="lt", bufs=2) as ltp,
            tc.tile_pool(name="small", bufs=3) as small,
            tc.tile_pool(name="outp", bufs=3) as outp,
            tc.tile_pool(name="psum", bufs=2, space="PSUM") as psum,
            tc.tile_pool(name="psmm", bufs=2, space="PSUM") as psmm,
            tc.tile_pool(name="pjunk", bufs=2) as pjunk,
        ):
            ident = const.tile([P, P], f16, tag="ident")
            nc.sync.dma_start(out=ident[:], in_=identw[:])
            ones1 = const.tile([1, P], f16, tag="ones")
            nc.sync.dma_start(out=ones1[:], in_=onesw[:])
            w1sb = const.tile([P, EC, D], f16, tag="w1")
            w2sb = const.tile([P, EC, D], f16, tag="w2")
            for ec in range(EC):
                nc.sync.dma_start(out=w1sb[:, ec, :], in_=w1t[ec * P:(ec + 1) * P, :])
                nc.sync.dma_start(out=w2sb[:, ec, :], in_=w2t[ec * P:(ec + 1) * P, :])
            bsb = const.tile([1, D], f16, tag="bias")
            nc.sync.dma_start(out=bsb[:], in_=biasw[:])
            ltm = const.tile([P, N * N], f32, tag="ltm")
            nc.sync.dma_start(out=ltm[:], in_=ltmw[:])

            st = [dict() for _ in range(nblk)]

            A_SPLIT = 8

            def phase_a1(blk):
                """loads + first chunk of neighbor-feature gathers + sims."""
                s = st[blk]
                r0, rp = blocks[blk]
                idx_t = small.tile([rp, N], i32, tag="idx")
                nc.sync.dma_start(out=idx_t[:], in_=nbrL[r0:r0 + rp, :])
                L_t = ltp.tile([rp, BD], f16, tag="lt")
                nc.sync.dma_start(out=L_t[:], in_=featsL[r0:r0 + rp, :])
                BLT = bltp.tile([P, EC * B * P], f16, tag="blt")
                nc.sync.dma_start(out=BLT[:], in_=bcastLT[blk * P:(blk + 1) * P, :])
                sims = small.tile([rp, N], f32, tag="sims")
                s.update(idx_t=idx_t, sims=sims, L_t=L_t, BLT=BLT,
                         r0=r0, rp=rp)
                gather_sims(blk, range(A_SPLIT))

            def gather_sims(blk, ns):
                s = st[blk]
                rp = s["rp"]
                for n in ns:
                    G = big.tile([rp, BD], f16, tag="big")
                    nc.gpsimd.indirect_dma_start(
                        out=G[:], out_offset=None, in_=feats[:],
                        in_offset=bass.IndirectOffsetOnAxis(
                            ap=s["idx_t"][:, n:n + 1], axis=0),
                    )
                    # product goes to PSUM scratch (keeps SBUF write ports
                    # free for the concurrent gather DMAs); one full-row
                    # fused multiply+accumulate per candidate -> sims col.
                    junkp = pjunk.tile([P, BD], f16, tag="junk")
                    nc.vector.scalar_tensor_tensor(
                        out=junkp[:rp], in0=G[:],
                        scalar=0.0, in1=s["L_t"][:],
                        op0=Alu.bypass, op1=Alu.mult,
                        accum_out=s["sims"][:, n:n + 1],
                    )

            def phase_a2(blk):
                """remaining gathers + sims."""
                gather_sims(blk, range(A_SPLIT, N))

            def phase_b(blk):
                """exact stable rank (jax.lax.top_k tie-break) -> selected idx."""
                s = st[blk]
                sims, idx_t, rp = s["sims"], s["idx_t"], s["rp"]
                nbrf = small.tile([rp, N], f32, tag="nbrf")
                nc.vector.tensor_copy(out=nbrf[:], in_=idx_t[:])
                cnt = small.tile([rp, N], f32, tag="cnt")
                cnteq = small.tile([rp, N], f32, tag="cnteq")
                junk16 = small.tile([rp, N], f32, tag="junk16")
                # full 16x16 comparison matrices via stride-0 broadcast APs:
                # cmp[r, p, q] = op(sims[r, q], sims[r, p]); row-sum over q.
                simq = sims[:, None, :].to_broadcast([rp, N, N])
                simp = sims[:, :, None].to_broadcast([rp, N, N])
                cmp_t = small.tile([rp, N * N], f32, tag="cmp")
                cmpv = cmp_t[:].rearrange("p (a b) -> p a b", a=N)
                nc.vector.tensor_tensor(out=cmpv, in0=simq, in1=simp,
                                        op=Alu.is_gt)
                nc.vector.tensor_reduce(out=cnt[:], in_=cmpv,
                                        axis=mybir.AxisListType.X, op=Alu.add)
                nc.vector.tensor_tensor(out=cmpv, in0=simq, in1=simp,
                                        op=Alu.is_equal)
                nc.vector.tensor_tensor(out=cmpv, in0=cmpv,
                                        in1=ltm[:rp].rearrange(
                                            "p (a b) -> p a b", a=N),
                                        op=Alu.mult)
                nc.vector.tensor_reduce(out=cnteq[:], in_=cmpv,
                                        axis=mybir.AxisListType.X, op=Alu.add)
                rank = small.tile([rp, N], f32, tag="rank")
                nc.vector.tensor_tensor(out=rank[:], in0=cnt[:], in1=cnteq[:],
                                        op=Alu.add)
                # sel_k = neighbor index whose rank == k (unique by construction)
                self_f = small.tile([rp, TOP_K], f32, tag="self")
                for k in range(TOP_K):
                    nc.vector.scalar_tensor_tensor(
                        out=junk16[:], in0=rank[:], scalar=float(k), in1=nbrf[:],
                        op0=Alu.is_equal, op1=Alu.mult,
                        accum_out=self_f[:, k:k + 1],
                    )
                sel_i = small.tile([rp, TOP_K], i32, tag="seli")
                nc.vector.tensor_copy(out=sel_i[:], in_=self_f[:])
                s.update(sel_i=sel_i)

            def phase_cg(blk):
                """gather the 4 selected bcast rows (descgen enqueued early,
                while the current block's sims STT stream covers the ring
                latency)."""
                s = st[blk]
                sel_i, rp = s["sel_i"], s["rp"]
                g40 = big.tile([rp, BD], f16, tag="big")
                g41 = big.tile([rp, BD], f16, tag="big")
                g42 = big.tile([rp, BD], f16, tag="big")
                g43 = big.tile([rp, BD], f16, tag="big")
                G4 = [g40, g41, g42, g43]
                for k in range(TOP_K):
                    nc.gpsimd.indirect_dma_start(
                        out=G4[k][:], out_offset=None, in_=bcast[:],
                        in_offset=bass.IndirectOffsetOnAxis(
                            ap=sel_i[:, k:k + 1], axis=0),
                    )
                s.update(G4=G4)

            def phase_ca(blk):
                """sum the gathered rows (0.25 folded in w2t); emitted after a
                full STT stream so the gathers have long drained the ring and
                DVE never waits here."""
                s = st[blk]
                G4 = s["G4"]
                AG = G4[0]
                for k in range(1, TOP_K):
                    nc.vector.tensor_tensor(out=AG[:], in0=AG[:],
                                            in1=G4[k][:], op=Alu.add)
                s.update(AG=AG)

            def phase_d(blk):
                """transpose agg, final mix matmuls + bias, write out."""
                s = st[blk]
                r0, AG, BLT, rp = s["r0"], s["AG"], s["BLT"], s["rp"]
                if debug:
                    nc.sync.dma_start(out=dbg_sims[r0:r0 + P, :], in_=s["sims"][:])
                    nc.sync.dma_start(out=dbg_sel[r0:r0 + P, :], in_=s["sel_i"][:])
                AGT = big.tile([P, EC * rp * B], f16, tag="big")
                agv = AGT[:].rearrange("p (ec b r) -> p ec b r", ec=EC, b=B)
                outv = outL[:].rearrange("(r b) d -> r b d", b=B)
                # per-b interleave: each b's transposes feed its own matmul
                # chain immediately, so the first out-store fires ~14 b's
                # earlier than with a transpose-all-then-matmul-all order.
                for b_i in range(B):
                    for ec in range(EC):
                        pt = psum.tile([P, P], f16, tag="tr")
                        off = b_i * D + ec * P
                        nc.tensor.transpose(out=pt[:, :rp],
                                            in_=AG[:rp, off:off + P],
                                            identity=ident[:rp, :rp])
                        nc.scalar.copy(out=agv[:, ec, b_i, :],
                                       in_=pt[:, :rp])
                    ps = psmm.tile([P, D], f32, tag="mm")
                    first = True
                    for srcT, wsb in ((BLT, w1sb), (AGT, w2sb)):
                        for ec in range(EC):
                            off = ec * (B * rp) + b_i * rp
                            lhsT = srcT[:, off:off + rp]
                            nc.tensor.matmul(out=ps[:rp], lhsT=lhsT,
                                             rhs=wsb[:, ec, :],
                                             start=first, stop=False)
                            first = False
                    nc.tensor.matmul(out=ps[:rp], lhsT=ones1[:1, :rp],
                                     rhs=bsb[:1, :], start=False, stop=True)
                    ot = outp.tile([P, D], f16, tag="ot")
                    nc.scalar.copy(out=ot[:rp], in_=ps[:rp])
                    # out-stores ride the ACT HWDGE ring so they never sit in
                    # front of the next block's loads on the sync ring.
                    nc.scalar.dma_start(out=outv[r0:r0 + rp, b_i, :],
                                        in_=ot[:rp])

            # software-pipelined emission, C/D lag two blocks behind A/B so
            # the bcast-gather descgen never waits on sel_i (which is long
            # ready), and the gpsimd queue stays saturated with feats gathers.
            ph = {"a1": phase_a1, "a2": phase_a2, "b": phase_b,
                  "cg": phase_cg, "ca": phase_ca, "d": phase_d}
            sched = [("a1", 0), ("a2", 0), ("b", 0),
                     ("a1", 1), ("a2", 1), ("b", 1)]
            for b in range(2, nblk - 1):
                sched += [("a1", b), ("cg", b - 2), ("a2", b), ("ca", b - 2),
                          ("b", b), ("cg", b - 1), ("d", b - 2)]
            # cg(b-1) already rides at the end of the previous segment, so the
            # last A/B segment only adds cg(lb)'s ring load; the epilogue is a
            # single C+D.
            lb = nblk - 1
            sched += [("a1", lb), ("cg", lb - 1), ("a2", lb), ("ca", lb - 2),
                      ("b", lb), ("ca", lb - 1), ("d", lb - 2),
                      ("d", lb - 1),
                      ("cg", lb), ("ca", lb), ("d", lb)]
            for name, b in sched:
                ph[name](b)

    nc.compile()
    return nc


_CACHE = {}


def _get_nc():
    if "nc" not in _CACHE:
        _CACHE["nc"] = build()
    return _CACHE["nc"]


def _prep_in_maps(bcast_by_region, feats_by_region, neighbor_indices, mix_w,
                  mix_b):
    f2 = np.ascontiguousarray(
        np.asarray(feats_by_region, dtype=np.float32).reshape(R, BD)
    ).astype(np.float16)
    bc = np.ascontiguousarray(
        np.asarray(bcast_by_region, dtype=np.float32).reshape(R, BD)
    ).astype(np.float16)
    nbr = np.ascontiguousarray(np.asarray(neighbor_indices, dtype=np.int32))
    mw = np.asarray(mix_w, dtype=np.float32)
    mb = np.asarray(mix_b, dtype=np.float32)
    w1t = np.ascontiguousarray(mw[:, :D].T).astype(np.float16)
    w2t = (np.ascontiguousarray(mw[:, D:].T) * np.float32(1.0 / TOP_K)).astype(
        np.float16)
    biasw = np.ascontiguousarray(mb.reshape(1, D)).astype(np.float16)
    identw = np.eye(P, dtype=np.float16)
    onesw = np.ones((1, P), dtype=np.float16)
    # strict lower-triangular [a, b] -> 1 if b < a, replicated per partition
    tri = np.tril(np.ones((N, N), dtype=np.float32), -1).reshape(1, N * N)
    ltmw = np.ascontiguousarray(np.broadcast_to(tri, (P, N * N)))

    rl = R // NCORES
    nblk = rl // P
    in_maps = []
    for c in range(NCORES):
        bcl = bc[c * rl:(c + 1) * rl]
        # [blk, p, (ec, b, r)] layout for the matmul lhsT side
        bclt = np.ascontiguousarray(
            bcl.reshape(nblk, P, B, EC, P).transpose(0, 4, 3, 2, 1)
            .reshape(nblk * P, EC * B * P))
        in_maps.append({
            "feats": f2,
            "bcast": bc,
            "feats_local": np.ascontiguousarray(f2[c * rl:(c + 1) * rl]),
            "bcast_lt": bclt,
            "nbr_local": np.ascontiguousarray(nbr[c * rl:(c + 1) * rl]),
            "w1t": w1t,
            "w2t": w2t,
            "biasw": biasw,
            "identw": identw,
            "onesw": onesw,
            "ltmw": ltmw,
        })
    return in_maps


def run(in_maps, **kwargs):
    from concourse.bass_utils import run_bass_kernel_spmd

    nc = _get_nc()
    return run_bass_kernel_spmd(nc, in_maps, list(range(NCORES)), **kwargs)


def assemble(res):
    rl = R // NCORES
    return np.concatenate(
        [res.results[c]["out_local"].reshape(rl, B, D).astype(np.float32)
         for c in range(NCORES)],
        axis=0)


def kernel(bcast_by_region, feats_by_region, neighbor_indices, mix_w, mix_b):
    import os

    in_maps = _prep_in_maps(bcast_by_region, feats_by_region,
                            neighbor_indices, mix_w, mix_b)
    # NTFF tracing needs hooks this environment may not have; make sure a
    # stray BASS_TRACE env var can't break the plain execution path.
    prev = os.environ.get("BASS_NEVER_TRACE")
    os.environ["BASS_NEVER_TRACE"] = "1"
    try:
        res = run(in_maps)
    finally:
        if prev is None:
            os.environ.pop("BASS_NEVER_TRACE", None)
        else:
            os.environ["BASS_NEVER_TRACE"] = prev
    return assemble(res)
